# revision 16
# baseline (speedup 1.0000x reference)
"""Trainium2 Bass kernel for the entity-assignment loss.

Math: per sample b, C[i,j] = mean_d (yt[b,i,d]-yp[b,j,d])^2.
loss = mean_b ( min_perm sum_i C[i, perm(i)] / 8 ).

Since each permutation uses every row i and every column j exactly once,
  sum_i C[i, perm(i)] = (nt + np - 2 * sum_i dot(i, perm(i))) / 64
with nt = sum_i |yt_i|^2, np = sum_j |yp_j|^2 (per-sample constants).
So min over perms only needs MAX over perms of the dot sum, computed with a
2^8 bitmask DP whose bit-i update is a perfectly strided access pattern.

Sharding: pure data parallelism, 256 samples per core across 8 cores; the
final mean is taken on the host from per-sample partial results.
"""

import os
import sys

if "/opt/trn_rl_repo" not in sys.path:
    sys.path.insert(0, "/opt/trn_rl_repo")

import numpy as np

SPLIT = os.environ.get("K_SPLIT", "0") == "1"
RAW = os.environ.get("K_RAW", "0") == "1"
F16 = os.environ.get("K_F16", "1") == "1"
SQS = os.environ.get("K_SQS", "1") == "1"  # squares-on-ScalarE scheme

B, N, D = 2048, 8, 64
N_CORES = 8
B_LOC = B // N_CORES        # 256 samples per core
N_TILES = B_LOC // 128      # 2 partition tiles of 128 samples
NEG = -1.0e30

TRACE = False
_CACHE = {}


def _build_raw():
    """Raw bacc build: one DVE instruction stream + DMA on SyncE + norms on
    ScalarE, with only a handful of semaphores. Avoids Tile's per-op event
    semaphores and its ~11us exit barrier."""
    import concourse.bacc as bacc
    import concourse.mybir as mybir

    f32 = mybir.dt.float32
    Alu = mybir.AluOpType
    Act = mybir.ActivationFunctionType

    nc = bacc.Bacc("TRN2", target_bir_lowering=False, debug=False)
    yt_d = nc.declare_dram_parameter("yt", [B_LOC, N * D], f32, isOutput=False)
    yp_d = nc.declare_dram_parameter("yp", [B_LOC, N * D], f32, isOutput=False)
    out_d = nc.declare_dram_parameter("out", [128, N_TILES], f32, isOutput=True)

    with (
        nc.sbuf_tensor("yt0", [128, N * D], f32) as yt0,
        nc.sbuf_tensor("yp0", [128, N * D], f32) as yp0,
        nc.sbuf_tensor("yt1", [128, N * D], f32) as yt1,
        nc.sbuf_tensor("yp1", [128, N * D], f32) as yp1,
        nc.sbuf_tensor("prod", [128, N * N * D], f32) as prod,
        nc.sbuf_tensor("G", [128, N * N], f32) as G,
        nc.sbuf_tensor("dpa", [128, 256], f32) as dpa,
        nc.sbuf_tensor("dpb", [128, 256], f32) as dpb,
        nc.sbuf_tensor("sq", [128, N * D], f32) as sq,
        nc.sbuf_tensor("nt", [128, N_TILES], f32) as nt,
        nc.sbuf_tensor("npt", [128, N_TILES], f32) as npt,
        nc.sbuf_tensor("loss", [128, N_TILES], f32) as loss,
        nc.semaphore("dma_sem") as dma_sem,
        nc.semaphore("norm_sem") as norm_sem,
        nc.semaphore("v_done") as v_done,
        nc.Block() as block,
    ):
        yts = [yt0, yt1]
        yps = [yp0, yp1]

        @block.sync
        def _(sync):
            for c in range(N_TILES):
                sync.dma_start(
                    out=yts[c][:, :], in_=yt_d[c * 128:(c + 1) * 128, :]
                ).then_inc(dma_sem, 16)
                sync.dma_start(
                    out=yps[c][:, :], in_=yp_d[c * 128:(c + 1) * 128, :]
                ).then_inc(dma_sem, 16)
            sync.wait_ge(v_done, N_TILES)
            sync.dma_start(out=out_d[:, :], in_=loss[:, :]).then_inc(dma_sem, 16)
            sync.wait_ge(dma_sem, (2 * N_TILES + 1) * 16)

        @block.scalar
        def _(scalar):
            for c in range(N_TILES):
                scalar.wait_ge(dma_sem, (c + 1) * 32)
                nc.scalar.activation(
                    out=sq[:, :], in_=yts[c][:, :], func=Act.Square,
                    accum_out=nt[:, c:c + 1])
                nc.scalar.activation(
                    out=sq[:, :], in_=yps[c][:, :], func=Act.Square,
                    accum_out=npt[:, c:c + 1]).then_inc(norm_sem, 1)

        @block.vector
        def _(vector):
            for c in range(N_TILES):
                yt_t, yp_t = yts[c], yps[c]
                vector.wait_ge(dma_sem, (c + 1) * 32)
                yt_b = yt_t.rearrange("p (i d) -> p i d", d=D).unsqueeze(2) \
                    .broadcast_to([128, N, N, D])
                yp_b = yp_t.rearrange("p (j d) -> p j d", d=D).unsqueeze(1) \
                    .broadcast_to([128, N, N, D])
                nc.vector.tensor_tensor(
                    out=prod.rearrange("p (i j d) -> p i j d", j=N, d=D),
                    in0=yt_b, in1=yp_b, op=Alu.mult)
                nc.vector.tensor_reduce(
                    out=G[:, :],
                    in_=prod.rearrange("p (q d) -> p q d", d=D),
                    axis=mybir.AxisListType.X, op=Alu.add)

                nc.vector.memset(dpa[:, :], NEG)
                nc.vector.memset(dpb[:, :], NEG)
                nc.vector.memset(dpa[:, 0:1], 0.0)
                bufs = [dpa, dpb]
                for k in range(N):
                    old = bufs[k % 2]
                    new = bufs[(k + 1) % 2]
                    for i in range(N):
                        ci = 2 ** i
                        if SQS:
                            gcol = svecs[k][:, i:i + 1]
                        else:
                            gcol = G[:, i * N + k:i * N + k + 1]
                        if k == 0:
                            tgt = new[:, ci:ci + 1]
                            src = old[:, 0:1]
                        elif k == N - 1:
                            tgt = new[:, 255:256]
                            src = old[:, 255 - ci:256 - ci]
                        else:
                            vo = old.rearrange("p (a b c) -> p a b c", b=2, c=ci)
                            vn = new.rearrange("p (a b c) -> p a b c", b=2, c=ci)
                            tgt = vn[:, :, 1, :]
                            src = vo[:, :, 0, :]
                        nc.vector.scalar_tensor_tensor(
                            out=tgt, in0=src, scalar=G[:, col:col + 1],
                            in1=tgt, op0=Alu.add, op1=Alu.max)
                final = bufs[N % 2]
                vector.wait_ge(norm_sem, c + 1)
                nc.vector.tensor_add(
                    nt[:, c:c + 1], nt[:, c:c + 1], npt[:, c:c + 1])
                nc.vector.scalar_tensor_tensor(
                    out=loss[:, c:c + 1], in0=final[:, 255:256], scalar=-2.0,
                    in1=nt[:, c:c + 1], op0=Alu.mult,
                    op1=Alu.add).then_inc(v_done, 1)

    nc.compile()
    return nc


def _build():
    import concourse.bass as bass
    import concourse.bacc as bacc
    import concourse.mybir as mybir
    from concourse.tile import TileContext

    f32 = mybir.dt.float32
    f16 = mybir.dt.float16 if F16 else mybir.dt.float32
    neg = -60000.0 if F16 else NEG
    Alu = mybir.AluOpType
    Act = mybir.ActivationFunctionType

    nc = bacc.Bacc("TRN2", target_bir_lowering=False, debug=False)
    yt_d = nc.declare_dram_parameter("yt", [B_LOC, N * D], f32, isOutput=False)
    yp_d = nc.declare_dram_parameter("yp", [B_LOC, N * D], f32, isOutput=False)
    out_d = nc.declare_dram_parameter("out", [128, N_TILES], f32, isOutput=True)

    with TileContext(nc) as tc:
        with (
            tc.tile_pool(name="io", bufs=2) as io_pool,
            tc.tile_pool(name="work", bufs=2) as work_pool,
            tc.tile_pool(name="res", bufs=1) as res_pool,
        ):
            loss_t = res_pool.tile([128, N_TILES], f32, tag="loss")
            for c in range(N_TILES):
                # engine for this tile's elementwise work: with SPLIT, tile 1
                # runs its multiply + DP on GpSimd in parallel with DVE
                eng = nc.gpsimd if (SPLIT and c == 1) else nc.vector
                yt_t = io_pool.tile([128, N * D], f32, tag="yt")
                yp_t = io_pool.tile([128, N * D], f32, tag="yp")
                nc.sync.dma_start(out=yt_t[:, :], in_=yt_d[c * 128:(c + 1) * 128, :])
                nc.sync.dma_start(out=yp_t[:, :], in_=yp_d[c * 128:(c + 1) * 128, :])

                # Per-sample squared norms (ScalarE, overlapped with DVE work):
                # s = sum over all (i,d) of yt^2 + yp^2.
                sq = work_pool.tile([128, N * D], f32, tag="sq")
                nt = work_pool.tile([128, 1], f32, tag="nt")
                npt = work_pool.tile([128, 1], f32, tag="npt")
                s = work_pool.tile([128, 1], f32, tag="s")
                nc.scalar.activation(out=sq[:, :], in_=yt_t[:, :], func=Act.Square,
                                     accum_out=nt[:, 0:1])
                nc.scalar.activation(out=sq[:, :], in_=yp_t[:, :], func=Act.Square,
                                     accum_out=npt[:, 0:1])
                nc.vector.tensor_add(s[:, 0:1], nt[:, 0:1], npt[:, 0:1])
                if F16:
                    yth = work_pool.tile([128, N * D], f16, tag="yth")
                    yph = work_pool.tile([128, N * D], f16, tag="yph")
                    nc.scalar.activation(out=yth[:, :], in_=yt_t[:, :],
                                         func=Act.Identity)
                    nc.scalar.activation(out=yph[:, :], in_=yp_t[:, :],
                                         func=Act.Identity)
                    yt_t, yp_t = yth, yph

                # Pairwise terms. SQS scheme: DVE computes only the broadcast
                # sums yt_i + yp_j; the 64 segmented square-sums
                # s(i,j) = sum_d (yt_i+yp_j)^2 = nt_i + np_j + 2*dot(i,j)
                # run on the otherwise-idle ScalarE, ordered so DP stage k's
                # columns land first. The DP maxes s directly (monotone
                # shift); l_raw = 2*(nt+np) - S*.
                yt_b = yt_t.rearrange("p (i d) -> p i d", d=D).unsqueeze(2) \
                    .broadcast_to([128, N, N, D])
                yp_b = yp_t.rearrange("p (j d) -> p j d", d=D).unsqueeze(1) \
                    .broadcast_to([128, N, N, D])
                if SQS:
                    sums = work_pool.tile([128, N * N * D], f16, tag="prod")
                    eng.tensor_tensor(
                        out=sums.rearrange("p (i j d) -> p i j d", j=N, d=D),
                        in0=yt_b, in1=yp_b, op=Alu.add)
                    svecs = [work_pool.tile([128, N], f32, tag=f"sv{k}",
                                            name=f"sv{k}")
                             for k in range(N)]
                    sqs = work_pool.tile([128, D], f32, tag="sqs")
                    for k in range(N):
                        for i in range(N):
                            off = (i * N + k) * D
                            nc.scalar.activation(
                                out=sqs[:, :], in_=sums[:, off:off + D],
                                func=Act.Square,
                                accum_out=svecs[k][:, i:i + 1])
                else:
                    G = work_pool.tile([128, N * N], f32, tag="G")
                    prod = work_pool.tile([128, N * N * D], f16, tag="prod")
                    eng.tensor_tensor(
                        out=prod.rearrange("p (i j d) -> p i j d", j=N, d=D),
                        in0=yt_b, in1=yp_b, op=Alu.mult)
                    nc.vector.tensor_reduce(
                        out=G[:, :],
                        in_=prod.rearrange("p (q d) -> p q d", d=D),
                        axis=mybir.AxisListType.X, op=Alu.add)

                # Bitmask DP over row-subsets, one stage per column k.
                # dp[S] = best dot-sum assigning cols 0..k-1 to row set S.
                # Stale values in the ping-pong buffers encode injective
                # partial assignments on column subsets, which can never
                # reach the full state 255 early, so no per-stage reinit
                # is needed.
                dpa = work_pool.tile([128, 256], f16, tag="dpa")
                dpb = work_pool.tile([128, 256], f16, tag="dpb")
                eng.memset(dpa[:, :], neg)
                eng.memset(dpb[:, :], neg)
                eng.memset(dpa[:, 0:1], 0.0)
                bufs = [dpa, dpb]
                for k in range(N):
                    old = bufs[k % 2]
                    new = bufs[(k + 1) % 2]
                    for i in range(N):
                        ci = 2 ** i
                        if SQS:
                            gcol = svecs[k][:, i:i + 1]
                        else:
                            gcol = G[:, i * N + k:i * N + k + 1]
                        if k == 0:
                            # only source state 0 is live: write singletons
                            tgt = new[:, ci:ci + 1]
                            src = old[:, 0:1]
                        elif k == N - 1:
                            # only target state 255 matters
                            tgt = new[:, 255:256]
                            src = old[:, 255 - ci:256 - ci]
                        else:
                            vo = old.rearrange("p (a b c) -> p a b c", b=2, c=ci)
                            vn = new.rearrange("p (a b c) -> p a b c", b=2, c=ci)
                            tgt = vn[:, :, 1, :]
                            src = vo[:, :, 0, :]
                        eng.scalar_tensor_tensor(
                            out=tgt,
                            in0=src,
                            scalar=gcol,
                            in1=tgt,
                            op0=Alu.add,
                            op1=Alu.max,
                        )
                final = bufs[N % 2]

                # SQS: l_raw/2 = s - S*/2 (host divides by 256).
                # else: l_raw = nt + np - 2*D* (host divides by 512).
                eng.scalar_tensor_tensor(
                    out=loss_t[:, c:c + 1],
                    in0=final[:, 255:256],
                    scalar=-0.5 if SQS else -2.0,
                    in1=s[:, 0:1],
                    op0=Alu.mult,
                    op1=Alu.add,
                )
            nc.sync.dma_start(out=out_d[:, :], in_=loss_t[:, :])
    nc.compile()
    return nc


def kernel(y_true: np.ndarray, y_pred: np.ndarray) -> np.ndarray:
    from concourse.bass_utils import run_bass_kernel_spmd

    if "nc" not in _CACHE:
        _CACHE["nc"] = _build_raw() if RAW else _build()
    nc = _CACHE["nc"]

    yt = np.ascontiguousarray(np.asarray(y_true, dtype=np.float32)).reshape(B, N * D)
    yp = np.ascontiguousarray(np.asarray(y_pred, dtype=np.float32)).reshape(B, N * D)

    in_maps = [
        {
            "yt": np.ascontiguousarray(yt[c * B_LOC:(c + 1) * B_LOC]),
            "yp": np.ascontiguousarray(yp[c * B_LOC:(c + 1) * B_LOC]),
        }
        for c in range(N_CORES)
    ]
    res = run_bass_kernel_spmd(nc, in_maps, list(range(N_CORES)), trace=TRACE)
    _CACHE["last_results"] = res
    vals = np.concatenate([np.asarray(r["out"], dtype=np.float64).reshape(-1)
                           for r in res.results])
    loss = vals.mean() / ((D * N // 2) if (SQS and not RAW) else (D * N))
    return np.float32(loss)


# revision 18
# speedup vs baseline: 1.0209x; 1.0209x over previous
"""Trainium2 Bass kernel for the entity-assignment loss.

Math: per sample b, C[i,j] = mean_d (yt[b,i,d]-yp[b,j,d])^2.
loss = mean_b ( min_perm sum_i C[i, perm(i)] / 8 ).

Since each permutation uses every row i and every column j exactly once,
  sum_i C[i, perm(i)] = (nt + np - 2 * sum_i dot(i, perm(i))) / 64
with nt = sum_i |yt_i|^2, np = sum_j |yp_j|^2 (per-sample constants).
So min over perms only needs MAX over perms of the dot sum, computed with a
2^8 bitmask DP whose bit-i update is a perfectly strided access pattern.

Sharding: pure data parallelism, 256 samples per core across 8 cores; the
final mean is taken on the host from per-sample partial results.
"""

import os
import sys

if "/opt/trn_rl_repo" not in sys.path:
    sys.path.insert(0, "/opt/trn_rl_repo")

import numpy as np

SPLIT = os.environ.get("K_SPLIT", "0") == "1"
RAW = os.environ.get("K_RAW", "0") == "1"
F16 = os.environ.get("K_F16", "1") == "1"
SQS = os.environ.get("K_SQS", "0") == "1"
DPTT = os.environ.get("K_DPTT", "1") == "1"  # chunk-merged TT-based DP  # squares-on-ScalarE scheme

B, N, D = 2048, 8, 64
N_CORES = 8
B_LOC = B // N_CORES        # 256 samples per core
N_TILES = B_LOC // 128      # 2 partition tiles of 128 samples
NEG = -1.0e30

TRACE = False
_CACHE = {}


def _build_raw():
    """Raw bacc build: one DVE instruction stream + DMA on SyncE + norms on
    ScalarE, with only a handful of semaphores. Avoids Tile's per-op event
    semaphores and its ~11us exit barrier."""
    import concourse.bacc as bacc
    import concourse.mybir as mybir

    f32 = mybir.dt.float32
    Alu = mybir.AluOpType
    Act = mybir.ActivationFunctionType

    nc = bacc.Bacc("TRN2", target_bir_lowering=False, debug=False)
    yt_d = nc.declare_dram_parameter("yt", [B_LOC, N * D], f32, isOutput=False)
    yp_d = nc.declare_dram_parameter("yp", [B_LOC, N * D], f32, isOutput=False)
    out_d = nc.declare_dram_parameter("out", [128, N_TILES], f32, isOutput=True)

    with (
        nc.sbuf_tensor("yt0", [128, N * D], f32) as yt0,
        nc.sbuf_tensor("yp0", [128, N * D], f32) as yp0,
        nc.sbuf_tensor("yt1", [128, N * D], f32) as yt1,
        nc.sbuf_tensor("yp1", [128, N * D], f32) as yp1,
        nc.sbuf_tensor("prod", [128, N * N * D], f32) as prod,
        nc.sbuf_tensor("G", [128, N * N], f32) as G,
        nc.sbuf_tensor("dpa", [128, 256], f32) as dpa,
        nc.sbuf_tensor("dpb", [128, 256], f32) as dpb,
        nc.sbuf_tensor("sq", [128, N * D], f32) as sq,
        nc.sbuf_tensor("nt", [128, N_TILES], f32) as nt,
        nc.sbuf_tensor("npt", [128, N_TILES], f32) as npt,
        nc.sbuf_tensor("loss", [128, N_TILES], f32) as loss,
        nc.semaphore("dma_sem") as dma_sem,
        nc.semaphore("norm_sem") as norm_sem,
        nc.semaphore("v_done") as v_done,
        nc.Block() as block,
    ):
        yts = [yt0, yt1]
        yps = [yp0, yp1]

        @block.sync
        def _(sync):
            for c in range(N_TILES):
                sync.dma_start(
                    out=yts[c][:, :], in_=yt_d[c * 128:(c + 1) * 128, :]
                ).then_inc(dma_sem, 16)
                sync.dma_start(
                    out=yps[c][:, :], in_=yp_d[c * 128:(c + 1) * 128, :]
                ).then_inc(dma_sem, 16)
            sync.wait_ge(v_done, N_TILES)
            sync.dma_start(out=out_d[:, :], in_=loss[:, :]).then_inc(dma_sem, 16)
            sync.wait_ge(dma_sem, (2 * N_TILES + 1) * 16)

        @block.scalar
        def _(scalar):
            for c in range(N_TILES):
                scalar.wait_ge(dma_sem, (c + 1) * 32)
                nc.scalar.activation(
                    out=sq[:, :], in_=yts[c][:, :], func=Act.Square,
                    accum_out=nt[:, c:c + 1])
                nc.scalar.activation(
                    out=sq[:, :], in_=yps[c][:, :], func=Act.Square,
                    accum_out=npt[:, c:c + 1]).then_inc(norm_sem, 1)

        @block.vector
        def _(vector):
            for c in range(N_TILES):
                yt_t, yp_t = yts[c], yps[c]
                vector.wait_ge(dma_sem, (c + 1) * 32)
                yt_b = yt_t.rearrange("p (i d) -> p i d", d=D).unsqueeze(2) \
                    .broadcast_to([128, N, N, D])
                yp_b = yp_t.rearrange("p (j d) -> p j d", d=D).unsqueeze(1) \
                    .broadcast_to([128, N, N, D])
                nc.vector.tensor_tensor(
                    out=prod.rearrange("p (i j d) -> p i j d", j=N, d=D),
                    in0=yt_b, in1=yp_b, op=Alu.mult)
                nc.vector.tensor_reduce(
                    out=G[:, :],
                    in_=prod.rearrange("p (q d) -> p q d", d=D),
                    axis=mybir.AxisListType.X, op=Alu.add)

                nc.vector.memset(dpa[:, :], NEG)
                nc.vector.memset(dpb[:, :], NEG)
                nc.vector.memset(dpa[:, 0:1], 0.0)
                bufs = [dpa, dpb]
                for k in range(N):
                    old = bufs[k % 2]
                    new = bufs[(k + 1) % 2]
                    for i in range(N):
                        ci = 2 ** i
                        if SQS:
                            gcol = svecs[k][:, i:i + 1]
                        else:
                            gcol = G[:, i * N + k:i * N + k + 1]
                        if k == 0:
                            tgt = new[:, ci:ci + 1]
                            src = old[:, 0:1]
                        elif k == N - 1:
                            tgt = new[:, 255:256]
                            src = old[:, 255 - ci:256 - ci]
                        else:
                            vo = old.rearrange("p (a b c) -> p a b c", b=2, c=ci)
                            vn = new.rearrange("p (a b c) -> p a b c", b=2, c=ci)
                            tgt = vn[:, :, 1, :]
                            src = vo[:, :, 0, :]
                        nc.vector.scalar_tensor_tensor(
                            out=tgt, in0=src, scalar=G[:, col:col + 1],
                            in1=tgt, op0=Alu.add, op1=Alu.max)
                final = bufs[N % 2]
                vector.wait_ge(norm_sem, c + 1)
                nc.vector.tensor_add(
                    nt[:, c:c + 1], nt[:, c:c + 1], npt[:, c:c + 1])
                nc.vector.scalar_tensor_tensor(
                    out=loss[:, c:c + 1], in0=final[:, 255:256], scalar=-2.0,
                    in1=nt[:, c:c + 1], op0=Alu.mult,
                    op1=Alu.add).then_inc(v_done, 1)

    nc.compile()
    return nc


def _build():
    import concourse.bacc as bacc
    import concourse.mybir as mybir
    from concourse.tile import TileContext

    f32 = mybir.dt.float32
    f16 = mybir.dt.float16 if F16 else mybir.dt.float32
    neg = -60000.0 if F16 else NEG
    Alu = mybir.AluOpType
    Act = mybir.ActivationFunctionType
    NT = N_TILES

    nc = bacc.Bacc("TRN2", target_bir_lowering=False, debug=False)
    yt_d = nc.declare_dram_parameter("yt", [B_LOC, N * D], f32, isOutput=False)
    yp_d = nc.declare_dram_parameter("yp", [B_LOC, N * D], f32, isOutput=False)
    out_d = nc.declare_dram_parameter("out", [128, NT], f32, isOutput=True)

    with TileContext(nc) as tc:
        with (
            tc.tile_pool(name="io", bufs=2) as io_pool,
            tc.tile_pool(name="work", bufs=2) as work_pool,
            tc.tile_pool(name="res", bufs=1) as res_pool,
        ):
            loss_t = res_pool.tile([128, NT], f32, tag="loss")
            s_all = res_pool.tile([128, NT], f32, tag="s_all")
            G32 = res_pool.tile([128, NT * N * N], f32, tag="G32")
            for c in range(NT):
                yt_t = io_pool.tile([128, N * D], f32, tag="yt")
                yp_t = io_pool.tile([128, N * D], f32, tag="yp")
                nc.sync.dma_start(out=yt_t[:, :], in_=yt_d[c * 128:(c + 1) * 128, :])
                nc.sync.dma_start(out=yp_t[:, :], in_=yp_d[c * 128:(c + 1) * 128, :])

                # per-sample squared-norm totals on ScalarE (overlapped)
                sq = work_pool.tile([128, N * D], f32, tag="sq")
                nt = work_pool.tile([128, 1], f32, tag="nt")
                npt = work_pool.tile([128, 1], f32, tag="npt")
                nc.scalar.activation(out=sq[:, :], in_=yt_t[:, :], func=Act.Square,
                                     accum_out=nt[:, 0:1])
                nc.scalar.activation(out=sq[:, :], in_=yp_t[:, :], func=Act.Square,
                                     accum_out=npt[:, 0:1])
                nc.vector.tensor_add(s_all[:, c:c + 1], nt[:, 0:1], npt[:, 0:1])

                if F16:
                    yth = work_pool.tile([128, N * D], f16, tag="yth")
                    yph = work_pool.tile([128, N * D], f16, tag="yph")
                    nc.scalar.activation(out=yth[:, :], in_=yt_t[:, :],
                                         func=Act.Identity)
                    nc.scalar.activation(out=yph[:, :], in_=yp_t[:, :],
                                         func=Act.Identity)
                    yt_t, yp_t = yth, yph

                # dots: broadcast multiply, two binary folds over d, then
                # a 16-wide segmented reduce
                yt_b = yt_t.rearrange("p (i d) -> p i d", d=D).unsqueeze(2) \
                    .broadcast_to([128, N, N, D])
                yp_b = yp_t.rearrange("p (j d) -> p j d", d=D).unsqueeze(1) \
                    .broadcast_to([128, N, N, D])
                prod = work_pool.tile([128, N * N * D], f16, tag="prod")
                nc.vector.tensor_tensor(
                    out=prod.rearrange("p (i j d) -> p i j d", j=N, d=D),
                    in0=yt_b, in1=yp_b, op=Alu.mult)
                pv = prod.rearrange("p (q d) -> p q d", d=D)
                half = work_pool.tile([128, N * N * D // 2], f16, tag="half")
                hv = half.rearrange("p (q d) -> p q d", d=D // 2)
                nc.vector.tensor_tensor(
                    out=hv, in0=pv[:, :, 0:D // 2], in1=pv[:, :, D // 2:D],
                    op=Alu.add)
                quart = work_pool.tile([128, N * N * D // 4], f16, tag="quart")
                qv = quart.rearrange("p (q d) -> p q d", d=D // 4)
                nc.vector.tensor_tensor(
                    out=qv, in0=hv[:, :, 0:D // 4], in1=hv[:, :, D // 4:D // 2],
                    op=Alu.add)
                nc.vector.tensor_reduce(
                    out=G32[:, c * N * N:(c + 1) * N * N],
                    in_=qv, axis=mybir.AxisListType.X, op=Alu.add)

            # DP over both chunks jointly: states laid out [chunk, state]
            G16 = res_pool.tile([128, NT * N * N], f16, tag="G16")
            nc.vector.tensor_copy(G16[:, :], G32[:, :])
            g_v = G16.rearrange("p (h q) -> p h q", h=NT)

            dpa = res_pool.tile([128, NT * 256], f16, tag="dpa")
            dpb = res_pool.tile([128, NT * 256], f16, tag="dpb")
            dstep = 256
            nc.vector.memset(dpa[:, :], neg)
            nc.vector.memset(dpb[:, :], neg)
            for c in range(NT):
                nc.vector.memset(dpa[:, c * dstep:c * dstep + 1], 0.0)
            cand = res_pool.tile([128, NT * 128], f16, tag="cand")
            bufs = [dpa, dpb]
            for k in range(N):
                old = bufs[k % 2]
                new = bufs[(k + 1) % 2]
                for i in range(N):
                    ci = 2 ** i
                    col = i * N + k
                    gb1 = g_v[:, :, col:col + 1]
                    if k == 0:
                        src = old.rearrange("p (h s) -> p h s", h=NT)[:, :, 0:1]
                        tgt = new.rearrange("p (h s) -> p h s", h=NT)[:, :, ci:ci + 1]
                        cv = cand.rearrange("p (h s) -> p h s", h=NT)[:, :, 0:1]
                        gb = gb1
                    elif k == N - 1:
                        ov = old.rearrange("p (h s) -> p h s", h=NT)
                        src = ov[:, :, 255 - ci:256 - ci]
                        tgt = new.rearrange("p (h s) -> p h s", h=NT)[:, :, 255:256]
                        cv = cand.rearrange("p (h s) -> p h s", h=NT)[:, :, 0:1]
                        gb = gb1
                    else:
                        a = 256 // (2 * ci)
                        vo = old.rearrange("p (h a b c) -> p h a b c",
                                           h=NT, b=2, c=ci)
                        vn = new.rearrange("p (h a b c) -> p h a b c",
                                           h=NT, b=2, c=ci)
                        src = vo[:, :, :, 0, :]
                        tgt = vn[:, :, :, 1, :]
                        cv = cand.rearrange("p (h a c) -> p h a c",
                                            h=NT, c=ci)
                        gb = gb1.unsqueeze(3).broadcast_to([128, NT, a, ci])
                    nc.vector.tensor_tensor(out=cv, in0=src, in1=gb, op=Alu.add)
                    nc.vector.tensor_tensor(out=tgt, in0=tgt, in1=cv, op=Alu.max)
            final = bufs[N % 2].rearrange("p (h s) -> p h s", h=NT)

            for c in range(NT):
                nc.vector.scalar_tensor_tensor(
                    out=loss_t[:, c:c + 1],
                    in0=final[:, c, 255:256],
                    scalar=-2.0,
                    in1=s_all[:, c:c + 1],
                    op0=Alu.mult,
                    op1=Alu.add,
                )
            nc.sync.dma_start(out=out_d[:, :], in_=loss_t[:, :])
    nc.compile()
    return nc


def kernel(y_true: np.ndarray, y_pred: np.ndarray) -> np.ndarray:
    from concourse.bass_utils import run_bass_kernel_spmd

    if "nc" not in _CACHE:
        _CACHE["nc"] = _build_raw() if RAW else _build()
    nc = _CACHE["nc"]

    yt = np.ascontiguousarray(np.asarray(y_true, dtype=np.float32)).reshape(B, N * D)
    yp = np.ascontiguousarray(np.asarray(y_pred, dtype=np.float32)).reshape(B, N * D)

    in_maps = [
        {
            "yt": np.ascontiguousarray(yt[c * B_LOC:(c + 1) * B_LOC]),
            "yp": np.ascontiguousarray(yp[c * B_LOC:(c + 1) * B_LOC]),
        }
        for c in range(N_CORES)
    ]
    res = run_bass_kernel_spmd(nc, in_maps, list(range(N_CORES)), trace=TRACE)
    _CACHE["last_results"] = res
    vals = np.concatenate([np.asarray(r["out"], dtype=np.float64).reshape(-1)
                           for r in res.results])
    loss = vals.mean() / (D * N)
    return np.float32(loss)


# revision 19
# speedup vs baseline: 1.3026x; 1.2759x over previous
"""Trainium2 Bass kernel for the entity-assignment loss.

Math: per sample b, C[i,j] = mean_d (yt[b,i,d]-yp[b,j,d])^2.
loss = mean_b ( min_perm sum_i C[i, perm(i)] / 8 ).

Since each permutation uses every row i and every column j exactly once,
  sum_i C[i, perm(i)] = (nt + np - 2 * sum_i dot(i, perm(i))) / 64
with nt = sum_i |yt_i|^2, np = sum_j |yp_j|^2 (per-sample constants).
So min over perms only needs MAX over perms of the dot sum, computed with a
2^8 bitmask DP whose bit-i update is a perfectly strided access pattern.

Sharding: pure data parallelism, 256 samples per core across 8 cores; the
final mean is taken on the host from per-sample partial results.
"""

import os
import sys

if "/opt/trn_rl_repo" not in sys.path:
    sys.path.insert(0, "/opt/trn_rl_repo")

import numpy as np

SPLIT = os.environ.get("K_SPLIT", "0") == "1"
RAW = os.environ.get("K_RAW", "0") == "1"
F16 = os.environ.get("K_F16", "1") == "1"
SQS = os.environ.get("K_SQS", "0") == "1"
DPTT = os.environ.get("K_DPTT", "0") == "1"  # chunk-merged TT-based DP  # squares-on-ScalarE scheme

B, N, D = 2048, 8, 64
N_CORES = 8
B_LOC = B // N_CORES        # 256 samples per core
N_TILES = B_LOC // 128      # 2 partition tiles of 128 samples
NEG = -1.0e30

TRACE = False
_CACHE = {}


def _build_raw():
    """Raw bacc build: one DVE instruction stream + DMA on SyncE + norms on
    ScalarE, with only a handful of semaphores. Avoids Tile's per-op event
    semaphores and its ~11us exit barrier."""
    import concourse.bacc as bacc
    import concourse.mybir as mybir

    f32 = mybir.dt.float32
    Alu = mybir.AluOpType
    Act = mybir.ActivationFunctionType

    nc = bacc.Bacc("TRN2", target_bir_lowering=False, debug=False)
    yt_d = nc.declare_dram_parameter("yt", [B_LOC, N * D], f32, isOutput=False)
    yp_d = nc.declare_dram_parameter("yp", [B_LOC, N * D], f32, isOutput=False)
    out_d = nc.declare_dram_parameter("out", [128, N_TILES], f32, isOutput=True)

    with (
        nc.sbuf_tensor("yt0", [128, N * D], f32) as yt0,
        nc.sbuf_tensor("yp0", [128, N * D], f32) as yp0,
        nc.sbuf_tensor("yt1", [128, N * D], f32) as yt1,
        nc.sbuf_tensor("yp1", [128, N * D], f32) as yp1,
        nc.sbuf_tensor("prod", [128, N * N * D], f32) as prod,
        nc.sbuf_tensor("G", [128, N * N], f32) as G,
        nc.sbuf_tensor("dpa", [128, 256], f32) as dpa,
        nc.sbuf_tensor("dpb", [128, 256], f32) as dpb,
        nc.sbuf_tensor("sq", [128, N * D], f32) as sq,
        nc.sbuf_tensor("nt", [128, N_TILES], f32) as nt,
        nc.sbuf_tensor("npt", [128, N_TILES], f32) as npt,
        nc.sbuf_tensor("loss", [128, N_TILES], f32) as loss,
        nc.semaphore("dma_sem") as dma_sem,
        nc.semaphore("norm_sem") as norm_sem,
        nc.semaphore("v_done") as v_done,
        nc.Block() as block,
    ):
        yts = [yt0, yt1]
        yps = [yp0, yp1]

        @block.sync
        def _(sync):
            for c in range(N_TILES):
                sync.dma_start(
                    out=yts[c][:, :], in_=yt_d[c * 128:(c + 1) * 128, :]
                ).then_inc(dma_sem, 16)
                sync.dma_start(
                    out=yps[c][:, :], in_=yp_d[c * 128:(c + 1) * 128, :]
                ).then_inc(dma_sem, 16)
            sync.wait_ge(v_done, N_TILES)
            sync.dma_start(out=out_d[:, :], in_=loss[:, :]).then_inc(dma_sem, 16)
            sync.wait_ge(dma_sem, (2 * N_TILES + 1) * 16)

        @block.scalar
        def _(scalar):
            for c in range(N_TILES):
                scalar.wait_ge(dma_sem, (c + 1) * 32)
                nc.scalar.activation(
                    out=sq[:, :], in_=yts[c][:, :], func=Act.Square,
                    accum_out=nt[:, c:c + 1])
                nc.scalar.activation(
                    out=sq[:, :], in_=yps[c][:, :], func=Act.Square,
                    accum_out=npt[:, c:c + 1]).then_inc(norm_sem, 1)

        @block.vector
        def _(vector):
            for c in range(N_TILES):
                yt_t, yp_t = yts[c], yps[c]
                vector.wait_ge(dma_sem, (c + 1) * 32)
                yt_b = yt_t.rearrange("p (i d) -> p i d", d=D).unsqueeze(2) \
                    .broadcast_to([128, N, N, D])
                yp_b = yp_t.rearrange("p (j d) -> p j d", d=D).unsqueeze(1) \
                    .broadcast_to([128, N, N, D])
                nc.vector.tensor_tensor(
                    out=prod.rearrange("p (i j d) -> p i j d", j=N, d=D),
                    in0=yt_b, in1=yp_b, op=Alu.mult)
                nc.vector.tensor_reduce(
                    out=G[:, :],
                    in_=prod.rearrange("p (q d) -> p q d", d=D),
                    axis=mybir.AxisListType.X, op=Alu.add)

                nc.vector.memset(dpa[:, :], NEG)
                nc.vector.memset(dpb[:, :], NEG)
                nc.vector.memset(dpa[:, 0:1], 0.0)
                bufs = [dpa, dpb]
                for k in range(N):
                    old = bufs[k % 2]
                    new = bufs[(k + 1) % 2]
                    for i in range(N):
                        ci = 2 ** i
                        if SQS:
                            gcol = svecs[k][:, i:i + 1]
                        else:
                            gcol = G[:, i * N + k:i * N + k + 1]
                        if k == 0:
                            tgt = new[:, ci:ci + 1]
                            src = old[:, 0:1]
                        elif k == N - 1:
                            tgt = new[:, 255:256]
                            src = old[:, 255 - ci:256 - ci]
                        else:
                            vo = old.rearrange("p (a b c) -> p a b c", b=2, c=ci)
                            vn = new.rearrange("p (a b c) -> p a b c", b=2, c=ci)
                            tgt = vn[:, :, 1, :]
                            src = vo[:, :, 0, :]
                        nc.vector.scalar_tensor_tensor(
                            out=tgt, in0=src, scalar=G[:, col:col + 1],
                            in1=tgt, op0=Alu.add, op1=Alu.max)
                final = bufs[N % 2]
                vector.wait_ge(norm_sem, c + 1)
                nc.vector.tensor_add(
                    nt[:, c:c + 1], nt[:, c:c + 1], npt[:, c:c + 1])
                nc.vector.scalar_tensor_tensor(
                    out=loss[:, c:c + 1], in0=final[:, 255:256], scalar=-2.0,
                    in1=nt[:, c:c + 1], op0=Alu.mult,
                    op1=Alu.add).then_inc(v_done, 1)

    nc.compile()
    return nc


def _build():
    import concourse.bacc as bacc
    import concourse.mybir as mybir
    from concourse.tile import TileContext

    f32 = mybir.dt.float32
    f16 = mybir.dt.float16 if F16 else mybir.dt.float32
    neg = -60000.0 if F16 else NEG
    Alu = mybir.AluOpType
    Act = mybir.ActivationFunctionType
    NT = N_TILES

    nc = bacc.Bacc("TRN2", target_bir_lowering=False, debug=False)
    yt_d = nc.declare_dram_parameter("yt", [B_LOC, N * D], f32, isOutput=False)
    yp_d = nc.declare_dram_parameter("yp", [B_LOC, N * D], f32, isOutput=False)
    out_d = nc.declare_dram_parameter("out", [128, NT], f32, isOutput=True)

    with TileContext(nc) as tc:
        with (
            tc.tile_pool(name="io", bufs=2) as io_pool,
            tc.tile_pool(name="work", bufs=2) as work_pool,
            tc.tile_pool(name="res", bufs=1) as res_pool,
        ):
            loss_t = res_pool.tile([128, NT], f32, tag="loss")
            s_all = res_pool.tile([128, NT], f32, tag="s_all")
            G32 = res_pool.tile([128, NT * N * N], f32, tag="G32")
            for c in range(NT):
                yt_t = io_pool.tile([128, N * D], f32, tag="yt")
                yp_t = io_pool.tile([128, N * D], f32, tag="yp")
                nc.sync.dma_start(out=yt_t[:, :], in_=yt_d[c * 128:(c + 1) * 128, :])
                nc.sync.dma_start(out=yp_t[:, :], in_=yp_d[c * 128:(c + 1) * 128, :])

                # per-sample squared-norm totals on ScalarE (overlapped)
                sq = work_pool.tile([128, N * D], f32, tag="sq")
                nt = work_pool.tile([128, 1], f32, tag="nt")
                npt = work_pool.tile([128, 1], f32, tag="npt")
                nc.scalar.activation(out=sq[:, :], in_=yt_t[:, :], func=Act.Square,
                                     accum_out=nt[:, 0:1])
                nc.scalar.activation(out=sq[:, :], in_=yp_t[:, :], func=Act.Square,
                                     accum_out=npt[:, 0:1])
                nc.vector.tensor_add(s_all[:, c:c + 1], nt[:, 0:1], npt[:, 0:1])

                if F16:
                    yth = work_pool.tile([128, N * D], f16, tag="yth")
                    yph = work_pool.tile([128, N * D], f16, tag="yph")
                    nc.scalar.activation(out=yth[:, :], in_=yt_t[:, :],
                                         func=Act.Identity)
                    nc.scalar.activation(out=yph[:, :], in_=yp_t[:, :],
                                         func=Act.Identity)
                    yt_t, yp_t = yth, yph

                # dots: broadcast multiply, two binary folds over d, then
                # a 16-wide segmented reduce
                yt_b = yt_t.rearrange("p (i d) -> p i d", d=D).unsqueeze(2) \
                    .broadcast_to([128, N, N, D])
                yp_b = yp_t.rearrange("p (j d) -> p j d", d=D).unsqueeze(1) \
                    .broadcast_to([128, N, N, D])
                prod = work_pool.tile([128, N * N * D], f16, tag="prod")
                nc.vector.tensor_tensor(
                    out=prod.rearrange("p (i j d) -> p i j d", j=N, d=D),
                    in0=yt_b, in1=yp_b, op=Alu.mult)
                pv = prod.rearrange("p (q d) -> p q d", d=D)
                half = work_pool.tile([128, N * N * D // 2], f16, tag="half")
                hv = half.rearrange("p (q d) -> p q d", d=D // 2)
                nc.vector.tensor_tensor(
                    out=hv, in0=pv[:, :, 0:D // 2], in1=pv[:, :, D // 2:D],
                    op=Alu.add)
                quart = work_pool.tile([128, N * N * D // 4], f16, tag="quart")
                qv = quart.rearrange("p (q d) -> p q d", d=D // 4)
                nc.vector.tensor_tensor(
                    out=qv, in0=hv[:, :, 0:D // 4], in1=hv[:, :, D // 4:D // 2],
                    op=Alu.add)
                nc.vector.tensor_reduce(
                    out=G32[:, c * N * N:(c + 1) * N * N],
                    in_=qv, axis=mybir.AxisListType.X, op=Alu.add)

            # DP over both chunks jointly: states laid out [chunk, state]
            if DPTT:
                G16 = res_pool.tile([128, NT * N * N], f16, tag="G16")
                nc.vector.tensor_copy(G16[:, :], G32[:, :])
                g_v = G16.rearrange("p (h q) -> p h q", h=NT)
            else:
                g_v = G32.rearrange("p (h q) -> p h q", h=NT)

            dpa = res_pool.tile([128, NT * 256], f16, tag="dpa")
            dpb = res_pool.tile([128, NT * 256], f16, tag="dpb")
            dstep = 256
            nc.vector.memset(dpa[:, :], neg)
            nc.vector.memset(dpb[:, :], neg)
            for c in range(NT):
                nc.vector.memset(dpa[:, c * dstep:c * dstep + 1], 0.0)
            cand = res_pool.tile([128, NT * 128], f16, tag="cand")
            bufs = [dpa, dpb]
            for k in range(N):
                old = bufs[k % 2]
                new = bufs[(k + 1) % 2]
                for i in range(N):
                    ci = 2 ** i
                    col = i * N + k
                    gb1 = g_v[:, :, col:col + 1]
                    if k == 0:
                        src = old.rearrange("p (h s) -> p h s", h=NT)[:, :, 0:1]
                        tgt = new.rearrange("p (h s) -> p h s", h=NT)[:, :, ci:ci + 1]
                        cv = cand.rearrange("p (h s) -> p h s", h=NT)[:, :, 0:1]
                        gb = gb1
                    elif k == N - 1:
                        ov = old.rearrange("p (h s) -> p h s", h=NT)
                        src = ov[:, :, 255 - ci:256 - ci]
                        tgt = new.rearrange("p (h s) -> p h s", h=NT)[:, :, 255:256]
                        cv = cand.rearrange("p (h s) -> p h s", h=NT)[:, :, 0:1]
                        gb = gb1
                    else:
                        a = 256 // (2 * ci)
                        vo = old.rearrange("p (h a b c) -> p h a b c",
                                           h=NT, b=2, c=ci)
                        vn = new.rearrange("p (h a b c) -> p h a b c",
                                           h=NT, b=2, c=ci)
                        src = vo[:, :, :, 0, :]
                        tgt = vn[:, :, :, 1, :]
                        cv = cand.rearrange("p (h a c) -> p h a c",
                                            h=NT, c=ci)
                        gb = gb1.unsqueeze(3).broadcast_to([128, NT, a, ci])
                    if DPTT:
                        nc.vector.tensor_tensor(out=cv, in0=src, in1=gb, op=Alu.add)
                        nc.vector.tensor_tensor(out=tgt, in0=tgt, in1=cv, op=Alu.max)
                    else:
                        for h in range(NT):
                            nc.vector.scalar_tensor_tensor(
                                out=tgt[:, h], in0=src[:, h],
                                scalar=G32[:, h * N * N + col:h * N * N + col + 1],
                                in1=tgt[:, h], op0=Alu.add, op1=Alu.max)
            final = bufs[N % 2].rearrange("p (h s) -> p h s", h=NT)

            for c in range(NT):
                nc.vector.scalar_tensor_tensor(
                    out=loss_t[:, c:c + 1],
                    in0=final[:, c, 255:256],
                    scalar=-2.0,
                    in1=s_all[:, c:c + 1],
                    op0=Alu.mult,
                    op1=Alu.add,
                )
            nc.sync.dma_start(out=out_d[:, :], in_=loss_t[:, :])
    nc.compile()
    return nc


def kernel(y_true: np.ndarray, y_pred: np.ndarray) -> np.ndarray:
    from concourse.bass_utils import run_bass_kernel_spmd

    if "nc" not in _CACHE:
        _CACHE["nc"] = _build_raw() if RAW else _build()
    nc = _CACHE["nc"]

    yt = np.ascontiguousarray(np.asarray(y_true, dtype=np.float32)).reshape(B, N * D)
    yp = np.ascontiguousarray(np.asarray(y_pred, dtype=np.float32)).reshape(B, N * D)

    in_maps = [
        {
            "yt": np.ascontiguousarray(yt[c * B_LOC:(c + 1) * B_LOC]),
            "yp": np.ascontiguousarray(yp[c * B_LOC:(c + 1) * B_LOC]),
        }
        for c in range(N_CORES)
    ]
    res = run_bass_kernel_spmd(nc, in_maps, list(range(N_CORES)), trace=TRACE)
    _CACHE["last_results"] = res
    vals = np.concatenate([np.asarray(r["out"], dtype=np.float64).reshape(-1)
                           for r in res.results])
    loss = vals.mean() / (D * N)
    return np.float32(loss)


# revision 20
# speedup vs baseline: 1.3238x; 1.0162x over previous
"""Trainium2 Bass kernel for the entity-assignment loss.

Math: per sample b, C[i,j] = mean_d (yt[b,i,d]-yp[b,j,d])^2.
loss = mean_b ( min_perm sum_i C[i, perm(i)] / 8 ).

Since each permutation uses every row i and every column j exactly once,
  sum_i C[i, perm(i)] = (nt + np - 2 * sum_i dot(i, perm(i))) / 64
with nt = sum_i |yt_i|^2, np = sum_j |yp_j|^2 (per-sample constants).
So min over perms only needs MAX over perms of the dot sum, computed with a
2^8 bitmask DP whose bit-i update is a perfectly strided access pattern.

Sharding: pure data parallelism, 256 samples per core across 8 cores; the
final mean is taken on the host from per-sample partial results.
"""

import os
import sys

if "/opt/trn_rl_repo" not in sys.path:
    sys.path.insert(0, "/opt/trn_rl_repo")

import numpy as np

SPLIT = os.environ.get("K_SPLIT", "0") == "1"
RAW = os.environ.get("K_RAW", "0") == "1"
F16 = os.environ.get("K_F16", "1") == "1"
SQS = os.environ.get("K_SQS", "0") == "1"
DPTT = os.environ.get("K_DPTT", "0") == "1"  # chunk-merged TT-based DP  # squares-on-ScalarE scheme

B, N, D = 2048, 8, 64
N_CORES = 8
B_LOC = B // N_CORES        # 256 samples per core
N_TILES = B_LOC // 128      # 2 partition tiles of 128 samples
NEG = -1.0e30

TRACE = False
_CACHE = {}


def _build_raw():
    """Raw bacc build: one DVE instruction stream + DMA on SyncE + norms on
    ScalarE, with only a handful of semaphores. Avoids Tile's per-op event
    semaphores and its ~11us exit barrier."""
    import concourse.bacc as bacc
    import concourse.mybir as mybir

    f32 = mybir.dt.float32
    Alu = mybir.AluOpType
    Act = mybir.ActivationFunctionType

    nc = bacc.Bacc("TRN2", target_bir_lowering=False, debug=False)
    yt_d = nc.declare_dram_parameter("yt", [B_LOC, N * D], f32, isOutput=False)
    yp_d = nc.declare_dram_parameter("yp", [B_LOC, N * D], f32, isOutput=False)
    out_d = nc.declare_dram_parameter("out", [128, N_TILES], f32, isOutput=True)

    with (
        nc.sbuf_tensor("yt0", [128, N * D], f32) as yt0,
        nc.sbuf_tensor("yp0", [128, N * D], f32) as yp0,
        nc.sbuf_tensor("yt1", [128, N * D], f32) as yt1,
        nc.sbuf_tensor("yp1", [128, N * D], f32) as yp1,
        nc.sbuf_tensor("prod", [128, N * N * D], f32) as prod,
        nc.sbuf_tensor("G", [128, N * N], f32) as G,
        nc.sbuf_tensor("dpa", [128, 256], f32) as dpa,
        nc.sbuf_tensor("dpb", [128, 256], f32) as dpb,
        nc.sbuf_tensor("sq", [128, N * D], f32) as sq,
        nc.sbuf_tensor("nt", [128, N_TILES], f32) as nt,
        nc.sbuf_tensor("npt", [128, N_TILES], f32) as npt,
        nc.sbuf_tensor("loss", [128, N_TILES], f32) as loss,
        nc.semaphore("dma_sem") as dma_sem,
        nc.semaphore("norm_sem") as norm_sem,
        nc.semaphore("v_done") as v_done,
        nc.Block() as block,
    ):
        yts = [yt0, yt1]
        yps = [yp0, yp1]

        @block.sync
        def _(sync):
            for c in range(N_TILES):
                sync.dma_start(
                    out=yts[c][:, :], in_=yt_d[c * 128:(c + 1) * 128, :]
                ).then_inc(dma_sem, 16)
                sync.dma_start(
                    out=yps[c][:, :], in_=yp_d[c * 128:(c + 1) * 128, :]
                ).then_inc(dma_sem, 16)
            sync.wait_ge(v_done, N_TILES)
            sync.dma_start(out=out_d[:, :], in_=loss[:, :]).then_inc(dma_sem, 16)
            sync.wait_ge(dma_sem, (2 * N_TILES + 1) * 16)

        @block.scalar
        def _(scalar):
            for c in range(N_TILES):
                scalar.wait_ge(dma_sem, (c + 1) * 32)
                nc.scalar.activation(
                    out=sq[:, :], in_=yts[c][:, :], func=Act.Square,
                    accum_out=nt[:, c:c + 1])
                nc.scalar.activation(
                    out=sq[:, :], in_=yps[c][:, :], func=Act.Square,
                    accum_out=npt[:, c:c + 1]).then_inc(norm_sem, 1)

        @block.vector
        def _(vector):
            for c in range(N_TILES):
                yt_t, yp_t = yts[c], yps[c]
                vector.wait_ge(dma_sem, (c + 1) * 32)
                yt_b = yt_t.rearrange("p (i d) -> p i d", d=D).unsqueeze(2) \
                    .broadcast_to([128, N, N, D])
                yp_b = yp_t.rearrange("p (j d) -> p j d", d=D).unsqueeze(1) \
                    .broadcast_to([128, N, N, D])
                nc.vector.tensor_tensor(
                    out=prod.rearrange("p (i j d) -> p i j d", j=N, d=D),
                    in0=yt_b, in1=yp_b, op=Alu.mult)
                nc.vector.tensor_reduce(
                    out=G[:, :],
                    in_=prod.rearrange("p (q d) -> p q d", d=D),
                    axis=mybir.AxisListType.X, op=Alu.add)

                nc.vector.memset(dpa[:, :], NEG)
                nc.vector.memset(dpb[:, :], NEG)
                nc.vector.memset(dpa[:, 0:1], 0.0)
                bufs = [dpa, dpb]
                for k in range(N):
                    old = bufs[k % 2]
                    new = bufs[(k + 1) % 2]
                    for i in range(N):
                        ci = 2 ** i
                        if SQS:
                            gcol = svecs[k][:, i:i + 1]
                        else:
                            gcol = G[:, i * N + k:i * N + k + 1]
                        if k == 0:
                            tgt = new[:, ci:ci + 1]
                            src = old[:, 0:1]
                        elif k == N - 1:
                            tgt = new[:, 255:256]
                            src = old[:, 255 - ci:256 - ci]
                        else:
                            vo = old.rearrange("p (a b c) -> p a b c", b=2, c=ci)
                            vn = new.rearrange("p (a b c) -> p a b c", b=2, c=ci)
                            tgt = vn[:, :, 1, :]
                            src = vo[:, :, 0, :]
                        nc.vector.scalar_tensor_tensor(
                            out=tgt, in0=src, scalar=G[:, col:col + 1],
                            in1=tgt, op0=Alu.add, op1=Alu.max)
                final = bufs[N % 2]
                vector.wait_ge(norm_sem, c + 1)
                nc.vector.tensor_add(
                    nt[:, c:c + 1], nt[:, c:c + 1], npt[:, c:c + 1])
                nc.vector.scalar_tensor_tensor(
                    out=loss[:, c:c + 1], in0=final[:, 255:256], scalar=-2.0,
                    in1=nt[:, c:c + 1], op0=Alu.mult,
                    op1=Alu.add).then_inc(v_done, 1)

    nc.compile()
    return nc


def _build():
    import concourse.bacc as bacc
    import concourse.mybir as mybir
    from concourse.tile import TileContext

    f32 = mybir.dt.float32
    f16 = mybir.dt.float16 if F16 else mybir.dt.float32
    neg = -60000.0 if F16 else NEG
    Alu = mybir.AluOpType
    Act = mybir.ActivationFunctionType
    NT = N_TILES

    nc = bacc.Bacc("TRN2", target_bir_lowering=False, debug=False)
    yt_d = nc.declare_dram_parameter("yt", [B_LOC, N * D], f32, isOutput=False)
    yp_d = nc.declare_dram_parameter("yp", [B_LOC, N * D], f32, isOutput=False)
    out_d = nc.declare_dram_parameter("out", [128, NT], f32, isOutput=True)

    with TileContext(nc) as tc:
        with (
            tc.tile_pool(name="io", bufs=2) as io_pool,
            tc.tile_pool(name="work", bufs=2) as work_pool,
            tc.tile_pool(name="res", bufs=1) as res_pool,
        ):
            loss_t = res_pool.tile([128, NT], f32, tag="loss")
            s_all = res_pool.tile([128, NT], f32, tag="s_all")
            G32 = res_pool.tile([128, NT * N * N], f32, tag="G32")
            for c in range(NT):
                yt_t = io_pool.tile([128, N * D], f32, tag="yt")
                yp_t = io_pool.tile([128, N * D], f32, tag="yp")
                nc.sync.dma_start(out=yt_t[:, :], in_=yt_d[c * 128:(c + 1) * 128, :])
                nc.sync.dma_start(out=yp_t[:, :], in_=yp_d[c * 128:(c + 1) * 128, :])

                # per-sample squared-norm totals on ScalarE (overlapped)
                sq = work_pool.tile([128, N * D], f32, tag="sq")
                nt = work_pool.tile([128, 1], f32, tag="nt")
                npt = work_pool.tile([128, 1], f32, tag="npt")
                nc.scalar.activation(out=sq[:, :], in_=yt_t[:, :], func=Act.Square,
                                     accum_out=nt[:, 0:1])
                nc.scalar.activation(out=sq[:, :], in_=yp_t[:, :], func=Act.Square,
                                     accum_out=npt[:, 0:1])
                nc.vector.tensor_add(s_all[:, c:c + 1], nt[:, 0:1], npt[:, 0:1])

                if F16:
                    yth = work_pool.tile([128, N * D], f16, tag="yth")
                    yph = work_pool.tile([128, N * D], f16, tag="yph")
                    nc.scalar.activation(out=yth[:, :], in_=yt_t[:, :],
                                         func=Act.Identity)
                    nc.scalar.activation(out=yph[:, :], in_=yp_t[:, :],
                                         func=Act.Identity)
                    yt_t, yp_t = yth, yph

                # dots: broadcast multiply, two binary folds over d, then
                # a 16-wide segmented reduce
                yt_b = yt_t.rearrange("p (i d) -> p i d", d=D).unsqueeze(2) \
                    .broadcast_to([128, N, N, D])
                yp_b = yp_t.rearrange("p (j d) -> p j d", d=D).unsqueeze(1) \
                    .broadcast_to([128, N, N, D])
                prod = work_pool.tile([128, N * N * D], f16, tag="prod")
                nc.vector.tensor_tensor(
                    out=prod.rearrange("p (i j d) -> p i j d", j=N, d=D),
                    in0=yt_b, in1=yp_b, op=Alu.mult)
                pv = prod.rearrange("p (q d) -> p q d", d=D)
                half = work_pool.tile([128, N * N * D // 2], f16, tag="half")
                hv = half.rearrange("p (q d) -> p q d", d=D // 2)
                nc.vector.tensor_tensor(
                    out=hv, in0=pv[:, :, 0:D // 2], in1=pv[:, :, D // 2:D],
                    op=Alu.add)
                quart = work_pool.tile([128, N * N * D // 4], f16, tag="quart")
                qv = quart.rearrange("p (q d) -> p q d", d=D // 4)
                nc.vector.tensor_tensor(
                    out=qv, in0=hv[:, :, 0:D // 4], in1=hv[:, :, D // 4:D // 2],
                    op=Alu.add)
                eighth = work_pool.tile([128, N * N * D // 8], f16, tag="eighth")
                ev = eighth.rearrange("p (q d) -> p q d", d=D // 8)
                nc.vector.tensor_tensor(
                    out=ev, in0=qv[:, :, 0:D // 8], in1=qv[:, :, D // 8:D // 4],
                    op=Alu.add)
                nc.vector.tensor_reduce(
                    out=G32[:, c * N * N:(c + 1) * N * N],
                    in_=ev, axis=mybir.AxisListType.X, op=Alu.add)

            # DP over both chunks jointly: states laid out [chunk, state]
            if DPTT:
                G16 = res_pool.tile([128, NT * N * N], f16, tag="G16")
                nc.vector.tensor_copy(G16[:, :], G32[:, :])
                g_v = G16.rearrange("p (h q) -> p h q", h=NT)
            else:
                g_v = G32.rearrange("p (h q) -> p h q", h=NT)

            dpa = res_pool.tile([128, NT * 256], f16, tag="dpa")
            dpb = res_pool.tile([128, NT * 256], f16, tag="dpb")
            dstep = 256
            nc.vector.memset(dpa[:, :], neg)
            nc.vector.memset(dpb[:, :], neg)
            for c in range(NT):
                nc.vector.memset(dpa[:, c * dstep:c * dstep + 1], 0.0)
            cand = res_pool.tile([128, NT * 128], f16, tag="cand")
            bufs = [dpa, dpb]
            for k in range(N):
                old = bufs[k % 2]
                new = bufs[(k + 1) % 2]
                for i in range(N):
                    ci = 2 ** i
                    col = i * N + k
                    gb1 = g_v[:, :, col:col + 1]
                    if k == 0:
                        src = old.rearrange("p (h s) -> p h s", h=NT)[:, :, 0:1]
                        tgt = new.rearrange("p (h s) -> p h s", h=NT)[:, :, ci:ci + 1]
                        cv = cand.rearrange("p (h s) -> p h s", h=NT)[:, :, 0:1]
                        gb = gb1
                    elif k == N - 1:
                        ov = old.rearrange("p (h s) -> p h s", h=NT)
                        src = ov[:, :, 255 - ci:256 - ci]
                        tgt = new.rearrange("p (h s) -> p h s", h=NT)[:, :, 255:256]
                        cv = cand.rearrange("p (h s) -> p h s", h=NT)[:, :, 0:1]
                        gb = gb1
                    else:
                        a = 256 // (2 * ci)
                        vo = old.rearrange("p (h a b c) -> p h a b c",
                                           h=NT, b=2, c=ci)
                        vn = new.rearrange("p (h a b c) -> p h a b c",
                                           h=NT, b=2, c=ci)
                        src = vo[:, :, :, 0, :]
                        tgt = vn[:, :, :, 1, :]
                        cv = cand.rearrange("p (h a c) -> p h a c",
                                            h=NT, c=ci)
                        gb = gb1.unsqueeze(3).broadcast_to([128, NT, a, ci])
                    if DPTT:
                        nc.vector.tensor_tensor(out=cv, in0=src, in1=gb, op=Alu.add)
                        nc.vector.tensor_tensor(out=tgt, in0=tgt, in1=cv, op=Alu.max)
                    else:
                        for h in range(NT):
                            nc.vector.scalar_tensor_tensor(
                                out=tgt[:, h], in0=src[:, h],
                                scalar=G32[:, h * N * N + col:h * N * N + col + 1],
                                in1=tgt[:, h], op0=Alu.add, op1=Alu.max)
            final = bufs[N % 2].rearrange("p (h s) -> p h s", h=NT)

            for c in range(NT):
                nc.vector.scalar_tensor_tensor(
                    out=loss_t[:, c:c + 1],
                    in0=final[:, c, 255:256],
                    scalar=-2.0,
                    in1=s_all[:, c:c + 1],
                    op0=Alu.mult,
                    op1=Alu.add,
                )
            nc.sync.dma_start(out=out_d[:, :], in_=loss_t[:, :])
    nc.compile()
    return nc


def kernel(y_true: np.ndarray, y_pred: np.ndarray) -> np.ndarray:
    from concourse.bass_utils import run_bass_kernel_spmd

    if "nc" not in _CACHE:
        _CACHE["nc"] = _build_raw() if RAW else _build()
    nc = _CACHE["nc"]

    yt = np.ascontiguousarray(np.asarray(y_true, dtype=np.float32)).reshape(B, N * D)
    yp = np.ascontiguousarray(np.asarray(y_pred, dtype=np.float32)).reshape(B, N * D)

    in_maps = [
        {
            "yt": np.ascontiguousarray(yt[c * B_LOC:(c + 1) * B_LOC]),
            "yp": np.ascontiguousarray(yp[c * B_LOC:(c + 1) * B_LOC]),
        }
        for c in range(N_CORES)
    ]
    res = run_bass_kernel_spmd(nc, in_maps, list(range(N_CORES)), trace=TRACE)
    _CACHE["last_results"] = res
    vals = np.concatenate([np.asarray(r["out"], dtype=np.float64).reshape(-1)
                           for r in res.results])
    loss = vals.mean() / (D * N)
    return np.float32(loss)
